# revision 11
# baseline (speedup 1.0000x reference)
"""AdmissibleStatesHead on 8 Trainium2 NeuronCores.

marginals[c] = segment_sum(softmax(E @ W.T + b), digit_c)  ==  P @ M_c
where M is a one-hot [N_VALID, 48] matrix built on host from valid_states.

Device work per core (valid-states sharded 8 ways, batch replicated):
  logits^T tile [128v, 512b] = sum_k wt[k,v].T @ et[k,b]   (fp8 DoubleRow PE, fp32 PSUM)
  exp tile = Exp(logits^T * inv_scale + bias)              (ScalarE, PSUM -> SBUF fp8)
  U^T [48, 512b] += M_chunk.T @ exp_tile                   (fp8 DoubleRow PE)
Host: sum per-core partials, normalize by concept-0 bucket sum (= softmax
denominator), reshape to [6, B, 8]. W is pre-scaled by a power of two into
fp8's range; the Exp activation's free affine undoes it.

Set KERNEL_BF16=1 for a bf16 fallback (~1.8x slower, ~15x more accurate);
KERNEL_TRACE=1 captures an NTFF profile and fills LAST_EXEC_NS.
"""

import os
import sys
import types

import numpy as np
import ml_dtypes

OUTCOMES = [8, 8, 8, 8, 8, 8]
N_TOTAL = 262144
N_VALID = 8192
B, D = 4096, 1024
N_CORES = 8
P = 128
V_S = N_VALID // N_CORES  # 1024 valid states per core
NK = D // P               # 8 contraction chunks
NV = V_S // P             # 8 v-tiles per core
NB = B // 512             # 8 batch tiles of 512
NJ = 48                   # 6 concepts x 8 outcomes

# W values are small (~N(0, 0.02^2) per spec); scale into fp8e4m3's normal
# range and undo the scale for free inside the Exp activation. Chosen per
# call from the data as a power of two; the compiled module is cached per
# scale value.
DEFAULT_W_SCALE = 64.0

USE_BF16 = bool(os.environ.get("KERNEL_BF16"))

LAST_EXEC_NS = None
LAST_RESULT = None
_compiled_cache = {}


def _pick_w_scale(wmax):
    import math

    if not np.isfinite(wmax) or wmax <= 0:
        return DEFAULT_W_SCALE
    # keep max|W*scale| around <=192 (fp8e4m3 max 448), scale a power of 2
    s = 2.0 ** math.floor(math.log2(192.0 / wmax))
    return float(min(max(s, 2.0 ** -10), 2.0 ** 20))


def _split_excess_waits(nc, limit=1):
    """This walrus build rejects instructions carrying more than ~1 sync-wait
    ("Too many sync wait commands"). Hoist excess waits onto injected NoOps
    right before the instruction on the same engine — sequencers are in-order,
    so the semantics are identical."""
    import concourse.mybir as mybir

    ctr = 0
    main_bb = nc.m.functions[0].blocks[0]
    stripped = []
    for ins in main_bb.instructions:
        nm = str(ins.name)
        op = ins.concise_opcode()
        if op == "Drain" or (op == "EventSemaphore" and nm.startswith("barrier_")):
            continue
        stripped.append(ins)
    main_bb.instructions = stripped
    for fn in nc.m.functions:
        for bb in fn.blocks:
            insts = bb.instructions
            new = []
            changed = False
            for ins in insts:
                si = ins.sync_info
                lim = 1 if ins.concise_opcode() == "Drain" else limit
                if si is not None and len(si.on_wait) > lim:
                    waits = list(si.on_wait)
                    for w in waits[:-lim]:
                        ctr += 1
                        nop = mybir.InstNoOp(name=f"waitsplit_{ctr}", ins=[], outs=[])
                        nop.engine = ins.engine
                        nop.sync_info = mybir.SyncInfo(on_update=[], on_wait=[w])
                        new.append(nop)
                    ins.sync_info = mybir.SyncInfo(
                        on_update=list(si.on_update), on_wait=waits[-lim:]
                    )
                    changed = True
                new.append(ins)
            if changed:
                bb.instructions = new


def _patch_tile_tail():
    import concourse.tile as tile
    from concourse.vector_clock import ScopedClock

    if getattr(tile.TileContext, "_tail_patched", False):
        return

    def _drain_and_barrier(self, tick_clock, wait_clock):
        drain_inst = self.nc.sync.drain()
        wait_clock.add_sem_waits(
            drain_inst.ins, ScopedClock({None: tick_clock.global_clock})
        )
        self.nc.all_engine_barrier()
        popped = self.nc._tile_sem_poison_stack.pop()
        assert popped is self._sem_poison
        self.nc.clear_and_free_semaphores(list(self.sems.allocated().values()))

    tile.TileContext._drain_and_barrier = _drain_and_barrier
    tile.TileContext._tail_patched = True


def _build_nc(w_scale):
    import concourse.bass as bass
    import concourse.mybir as mybir
    import concourse.tile as tile

    _patch_tile_tail()

    f32 = mybir.dt.float32
    bf16 = mybir.dt.bfloat16
    fp8 = mybir.dt.float8e4
    Exp = mybir.ActivationFunctionType.Exp

    in_dt = bf16 if USE_BF16 else fp8
    exp_scale = 1.0 if USE_BF16 else 1.0 / w_scale
    n_warm = int(os.environ.get("KERNEL_WARM", "4"))

    nc = bass.Bass()
    # wt is k-major so the first matmuls of batch tile 0 are gated by only
    # wt[:, 0:2] (256KB) + et tile 0's first k-pair (128KB).
    wt = nc.dram_tensor("wt", [P, NK, NV, P], in_dt, kind="ExternalInput")
    et = nc.dram_tensor("et", [P, NB, NK, 512], in_dt, kind="ExternalInput")
    mm = nc.dram_tensor("mm", [P, NV, NJ], in_dt, kind="ExternalInput")
    bias = nc.dram_tensor("bias", [P, NV], f32, kind="ExternalInput")
    # slot NB holds the second half-partial of batch tile NB-1 (the last
    # tile's segment matmul is split in two so the tail chain is shorter);
    # the host adds slots NB-1 and NB.
    out = nc.dram_tensor("out", [NJ, NB + 1, 512], bf16, kind="ExternalOutput")

    with (
        tile.TileContext(nc) as tc,
        tc.tile_pool(name="const", bufs=1) as cpool,
        tc.tile_pool(name="expp", bufs=3) as xpool,
        tc.tile_pool(name="ps", bufs=6, space="PSUM") as pspool,
        tc.tile_pool(name="ps2", bufs=2, space="PSUM") as ps2pool,
        tc.tile_pool(name="uo", bufs=2) as upool,
        tc.tile_pool(name="warm", bufs=1) as wpool,
    ):
        # Input DMA: all input triggers ride the SP HWDGE ring (the sync
        # sequencer is otherwise idle until the output stores; a trigger
        # costs ~0.6us of *sequencer* time, which would stall the
        # Activation exp pipeline if issued from there). The ring is FIFO
        # at packet granularity across the 16 SDMA engines, so order =
        # priority: the k-pair-0 weights and et tile 0 gate the first
        # matmuls; later et tiles are grouped into large-line chunks
        # (8-12KB/partition) to amortize the ~60-300ns per-packet overhead.
        wt_sb = cpool.tile([P, NK, NV, P], in_dt)
        et_sb = cpool.tile([P, NB, NK, 512], in_dt)
        b_sb = cpool.tile([P, NV], f32)
        m_sb = cpool.tile([P, NV, NJ], in_dt)
        nc.sync.dma_start(wt_sb[:, 0:2], wt[:, 0:2])
        nc.sync.dma_start(et_sb[:, 0, 0:2], et[:, 0, 0:2])
        nc.sync.dma_start(et_sb[:, 0, 2:], et[:, 0, 2:])
        nc.sync.dma_start(wt_sb[:, 2:4], wt[:, 2:4])
        nc.sync.dma_start(wt_sb[:, 4:6], wt[:, 4:6])
        nc.sync.dma_start(b_sb[:], bias[:])
        nc.sync.dma_start(wt_sb[:, 6:8], wt[:, 6:8])
        nc.sync.dma_start(m_sb[:], mm[:])
        nc.sync.dma_start(et_sb[:, 1:3], et[:, 1:3])
        nc.sync.dma_start(et_sb[:, 3:6], et[:, 3:6])
        nc.sync.dma_start(et_sb[:, 6:8], et[:, 6:8])
        et_tiles = [
            (et_sb[:, n, 0:2, :], et_sb[:, n, 2:, :]) for n in range(NB)
        ]

        # PE HAM warm-up: the clock gate only opens after ~3.4us of sustained
        # PE activity. A few throwaway matmuls on zeroed scratch bridge the
        # gap until the first input tiles land; batch tile 0's own matmuls
        # then finish the warm-up doing real work.
        warm_sb = wpool.tile([P, 512], in_dt)
        nc.gpsimd.memset(warm_sb[:], 0)
        warm_act = wpool.tile([P, 16], in_dt)
        # pull the ~2.7us Exp table load into the DMA-wait window
        nc.scalar.activation(warm_act[:], warm_sb[:, 0:16], Exp)
        # borrow a slot from the main psum pool; it returns to the
        # rotation once the warm-up matmuls are done
        warm_ps = pspool.tile([P, 512], f32, tag="ps")
        for _ in range(n_warm):
            nc.tensor.matmul(
                warm_ps[:],
                lhsT=warm_sb[:, 0:P],
                rhs=warm_sb[:],
                start=True,
                stop=True,
            )

        def lhsT_of(v, k):  # [128, 2, 128] k-pair weight slice (k-major wt)
            return wt_sb[:, k : k + 2, v]

        def emit_mm2_mms(exp_t, vlo, vhi, ups):
            for v in range(vlo, vhi, 2):
                nc.tensor.matmul(
                    ups[:],
                    lhsT=m_sb[:, v : v + 2, :],
                    rhs=exp_t[:, v : v + 2, :],
                    start=(v == vlo),
                    stop=(v == vhi - 2),
                    perf_mode=mybir.MatmulPerfMode.DoubleRow,
                )

        def emit_mm2(n, exp_t):
            ups = ps2pool.tile([NJ, 512], f32, tag="ups")
            if USE_BF16:
                for v in range(NV):
                    nc.tensor.matmul(
                        ups[:],
                        lhsT=m_sb[:, v, :],
                        rhs=exp_t[:, v, :],
                        start=(v == 0),
                        stop=(v == NV - 1),
                    )
            else:
                emit_mm2_mms(exp_t, 0, NV, ups)
            u_sb = upool.tile([NJ, 512], bf16, tag="u")
            nc.vector.tensor_copy(u_sb[:], ups[:])
            nc.sync.dma_start(out[:, n, :], u_sb[:])

        # Batch tile 0 runs k-pair-outer across all 8 PSUM banks: the first
        # matmuls need only wt k-pair 0 + et0 k-pair 0, so real work starts
        # ~2us earlier than a v-outer order which would gate on all of wt.
        exp0 = xpool.tile([P, NV, 512], in_dt, tag="exp")
        et_a0, et_b0 = et_tiles[0]
        if USE_BF16:
            ps0 = [
                pspool.tile([P, 512], f32, tag="ps", name=f"ps0_{i}")
                for i in range(6)
            ] + [
                ps2pool.tile([P, 512], f32, tag="ups", name=f"ps0_{6 + i}")
                for i in range(2)
            ]
            for k in range(NK):
                rhs = et_a0[:, k, :] if k < 2 else et_b0[:, k - 2, :]
                for v in range(NV):
                    nc.tensor.matmul(
                        ps0[v][:],
                        lhsT=wt_sb[:, k, v],
                        rhs=rhs,
                        start=(k == 0),
                        stop=(k == NK - 1),
                    )
        else:
            ps0 = [
                pspool.tile([P, 512], f32, tag="ps", name=f"ps0_{i}")
                for i in range(6)
            ] + [
                ps2pool.tile([P, 512], f32, tag="ups", name=f"ps0_{6 + i}")
                for i in range(2)
            ]
            for k in range(0, NK, 2):
                rhs = et_a0 if k == 0 else et_b0[:, k - 2 : k, :]
                for v in range(NV):
                    nc.tensor.matmul(
                        ps0[v][:],
                        lhsT=lhsT_of(v, k),
                        rhs=rhs,
                        start=(k == 0),
                        stop=(k == NK - 2),
                        perf_mode=mybir.MatmulPerfMode.DoubleRow,
                    )
        for v in range(NV):
            nc.scalar.activation(
                exp0[:, v, :], ps0[v][:], Exp, bias=b_sb[:, v : v + 1],
                scale=exp_scale,
            )

        # MM2 of tile n is deferred to tile n+1's v==6 slot: tile 0's exps
        # drain through ScalarE in a ~5.5us burst, so an earlier slot would
        # park the PE queue on an exp-complete wait.
        pending = (0, exp0)
        for n in range(1, NB):
            et_a, et_b = et_tiles[n]
            exp_t = xpool.tile([P, NV, 512], in_dt, tag="exp")
            for v in range(NV):
                ps = pspool.tile([P, 512], f32, tag="ps")
                if USE_BF16:
                    for k in range(NK):
                        rhs = et_a[:, k, :] if k < 2 else et_b[:, k - 2, :]
                        nc.tensor.matmul(
                            ps[:],
                            lhsT=wt_sb[:, k, v],
                            rhs=rhs,
                            start=(k == 0),
                            stop=(k == NK - 1),
                        )
                else:
                    for k in range(0, NK, 2):
                        rhs = et_a if k == 0 else et_b[:, k - 2 : k, :]
                        nc.tensor.matmul(
                            ps[:],
                            lhsT=lhsT_of(v, k),
                            rhs=rhs,
                            start=(k == 0),
                            stop=(k == NK - 2),
                            perf_mode=mybir.MatmulPerfMode.DoubleRow,
                        )
                nc.scalar.activation(
                    exp_t[:, v, :], ps[:], Exp, bias=b_sb[:, v : v + 1], scale=exp_scale
                )
                if pending is not None and v == 6:
                    emit_mm2(*pending)
                    pending = None
            pending = (n, exp_t)

        # Last tile's MM2 in two host-summed halves so the final chain after
        # the last exp is only 2 matmuls + a split cast + a 24KB store.
        n_last, exp_last = pending
        if USE_BF16:
            emit_mm2(n_last, exp_last)
        else:
            ups_a = ps2pool.tile([NJ, 512], f32, tag="ups")
            emit_mm2_mms(exp_last, 0, NV // 2, ups_a)
            u_a = upool.tile([NJ, 512], bf16, tag="u")
            nc.vector.tensor_copy(u_a[:], ups_a[:])
            nc.sync.dma_start(out[:, n_last, :], u_a[:])
            ups_b = ps2pool.tile([NJ, 512], f32, tag="ups")
            emit_mm2_mms(exp_last, NV // 2, NV, ups_b)
            u_b = upool.tile([NJ, 512], bf16, tag="u")
            # split the final PSUM->SBUF cast across DVE and ScalarE (both
            # idle by now) to halve the tail latency
            nc.vector.tensor_copy(u_b[:, 0:256], ups_b[:, 0:256])
            nc.scalar.activation(
                u_b[:, 256:], ups_b[:, 256:],
                mybir.ActivationFunctionType.Copy,
            )
            nc.sync.dma_start(out[:, NB, 0:256], u_b[:, 0:256])
            nc.sync.dma_start(out[:, NB, 256:], u_b[:, 256:])
    _split_excess_waits(nc)
    return nc


def _install_ntff_hook():
    """bass_utils' axon trace path imports antenv.axon_hooks, absent in this
    image; shim it using trn_boot's ctypes NTFF hook."""
    if "antenv.axon_hooks" in sys.modules:
        return
    try:
        from trn_agent_boot.trn_boot import _ntff_profile_via_ctypes

        hook = _ntff_profile_via_ctypes("/opt/axon/libaxon_pjrt.so")
    except Exception:
        hook = None
    mod = types.ModuleType("antenv.axon_hooks")
    mod.get_axon_ntff_profile_hook = lambda: hook
    sys.modules["antenv.axon_hooks"] = mod


def kernel(embeddings, W, b, valid_states):
    global LAST_EXEC_NS, LAST_RESULT
    E = np.asarray(embeddings, dtype=np.float32)
    Wf = np.asarray(W, dtype=np.float32)
    bf = np.asarray(b, dtype=np.float32)
    vs = np.asarray(valid_states).astype(np.int64)

    bf16 = ml_dtypes.bfloat16
    if USE_BF16:
        in_dt = bf16
        Wp = Wf
        w_scale = 1.0
    else:
        in_dt = ml_dtypes.float8_e4m3
        w_scale = _pick_w_scale(float(np.abs(Wf).max()))
        Wp = Wf * w_scale

    # et[p, n, k, j] = E[n*512+j, k*128+p]  (32KB contiguous per partition)
    Et = E.T.astype(in_dt)  # [D, B]
    et_host = np.ascontiguousarray(Et.reshape(NK, P, NB, 512).transpose(1, 2, 0, 3))

    # One-hot segment matrix M [N_VALID, 48]
    M = np.zeros((N_VALID, NJ), dtype=in_dt)
    stride = N_TOTAL
    for c, n_i in enumerate(OUTCOMES):
        stride //= n_i
        digit = (vs // stride) % n_i
        M[np.arange(N_VALID), c * 8 + digit] = 1

    in_maps = []
    for core in range(N_CORES):
        sl = slice(core * V_S, (core + 1) * V_S)
        # k-major: wt[p, k, v, q] = W[v*128+q, k*128+p] (2KB lines per k-pair)
        wt_host = np.ascontiguousarray(
            Wp[sl, :].T.astype(in_dt).reshape(NK, P, NV, P).transpose(1, 0, 2, 3)
        )
        m_host = np.ascontiguousarray(M[sl].reshape(NV, P, NJ).transpose(1, 0, 2))
        b_host = np.ascontiguousarray(bf[sl].reshape(NV, P).T)
        in_maps.append({"wt": wt_host, "et": et_host, "mm": m_host, "bias": b_host})

    from concourse.bass_utils import run_bass_kernel_spmd

    key = (USE_BF16, w_scale)
    if key not in _compiled_cache:
        _compiled_cache[key] = _build_nc(w_scale)
    nc_mod = _compiled_cache[key]

    kwargs = {}
    if os.environ.get("KERNEL_TRACE"):
        _install_ntff_hook()
        kwargs["trace"] = True

    res = run_bass_kernel_spmd(
        nc_mod, in_maps, core_ids=list(range(N_CORES)), **kwargs
    )
    LAST_EXEC_NS = res.exec_time_ns
    LAST_RESULT = res

    U = np.zeros((NJ, B), dtype=np.float64)
    for r in res.results:
        o = r["out"].astype(np.float64)  # [NJ, NB+1, 512]
        if not USE_BF16:
            o[:, NB - 1] += o[:, NB]  # second half-partial of the last tile
        U += o[:, :NB].reshape(NJ, B)
    denom = U[0:8].sum(axis=0)  # [B] total softmax denominator
    marg = U.reshape(6, 8, B) / denom  # [6, 8, B]
    return np.ascontiguousarray(marg.transpose(0, 2, 1)).astype(np.float32)



# revision 13
# speedup vs baseline: 1.0276x; 1.0276x over previous
"""AdmissibleStatesHead on 8 Trainium2 NeuronCores.

marginals[c] = segment_sum(softmax(E @ W.T + b), digit_c)  ==  P @ M_c
where M is a one-hot [N_VALID, 48] matrix built on host from valid_states.

Device work per core (valid-states sharded 8 ways, batch replicated):
  logits^T tile [128v, 512b] = sum_k wt[k,v].T @ et[k,b]   (fp8 DoubleRow PE, fp32 PSUM)
  exp tile = Exp(logits^T * inv_scale + bias)              (ScalarE, PSUM -> SBUF fp8)
  U^T [48, 512b] += M_chunk.T @ exp_tile                   (fp8 DoubleRow PE)
Host: sum per-core partials, normalize by concept-0 bucket sum (= softmax
denominator), reshape to [6, B, 8]. W is pre-scaled by a power of two into
fp8's range; the Exp activation's free affine undoes it.

Set KERNEL_BF16=1 for a bf16 fallback (~1.8x slower, ~15x more accurate);
KERNEL_TRACE=1 captures an NTFF profile and fills LAST_EXEC_NS.
"""

import os
import sys
import types

import numpy as np
import ml_dtypes

OUTCOMES = [8, 8, 8, 8, 8, 8]
N_TOTAL = 262144
N_VALID = 8192
B, D = 4096, 1024
N_CORES = 8
P = 128
V_S = N_VALID // N_CORES  # 1024 valid states per core
NK = D // P               # 8 contraction chunks
NV = V_S // P             # 8 v-tiles per core
NB = B // 512             # 8 batch tiles of 512
NJ = 48                   # 6 concepts x 8 outcomes

# W values are small (~N(0, 0.02^2) per spec); scale into fp8e4m3's normal
# range and undo the scale for free inside the Exp activation. Chosen per
# call from the data as a power of two; the compiled module is cached per
# scale value.
DEFAULT_W_SCALE = 64.0

USE_BF16 = bool(os.environ.get("KERNEL_BF16"))

LAST_EXEC_NS = None
LAST_RESULT = None
_compiled_cache = {}


def _pick_w_scale(wmax):
    import math

    if not np.isfinite(wmax) or wmax <= 0:
        return DEFAULT_W_SCALE
    # keep max|W*scale| around <=192 (fp8e4m3 max 448), scale a power of 2
    s = 2.0 ** math.floor(math.log2(192.0 / wmax))
    return float(min(max(s, 2.0 ** -10), 2.0 ** 20))


def _split_excess_waits(nc, limit=1):
    """This walrus build rejects instructions carrying more than ~1 sync-wait
    ("Too many sync wait commands"). Hoist excess waits onto injected NoOps
    right before the instruction on the same engine — sequencers are in-order,
    so the semantics are identical."""
    import concourse.mybir as mybir

    ctr = 0
    main_bb = nc.m.functions[0].blocks[0]
    stripped = []
    for ins in main_bb.instructions:
        nm = str(ins.name)
        op = ins.concise_opcode()
        if op == "Drain" or (op == "EventSemaphore" and nm.startswith("barrier_")):
            continue
        stripped.append(ins)
    main_bb.instructions = stripped
    for fn in nc.m.functions:
        for bb in fn.blocks:
            insts = bb.instructions
            new = []
            changed = False
            for ins in insts:
                si = ins.sync_info
                lim = 1 if ins.concise_opcode() == "Drain" else limit
                if si is not None and len(si.on_wait) > lim:
                    waits = list(si.on_wait)
                    for w in waits[:-lim]:
                        ctr += 1
                        nop = mybir.InstNoOp(name=f"waitsplit_{ctr}", ins=[], outs=[])
                        nop.engine = ins.engine
                        nop.sync_info = mybir.SyncInfo(on_update=[], on_wait=[w])
                        new.append(nop)
                    ins.sync_info = mybir.SyncInfo(
                        on_update=list(si.on_update), on_wait=waits[-lim:]
                    )
                    changed = True
                new.append(ins)
            if changed:
                bb.instructions = new


def _patch_tile_tail():
    import concourse.tile as tile
    from concourse.vector_clock import ScopedClock

    if getattr(tile.TileContext, "_tail_patched", False):
        return

    def _drain_and_barrier(self, tick_clock, wait_clock):
        drain_inst = self.nc.sync.drain()
        wait_clock.add_sem_waits(
            drain_inst.ins, ScopedClock({None: tick_clock.global_clock})
        )
        self.nc.all_engine_barrier()
        popped = self.nc._tile_sem_poison_stack.pop()
        assert popped is self._sem_poison
        self.nc.clear_and_free_semaphores(list(self.sems.allocated().values()))

    tile.TileContext._drain_and_barrier = _drain_and_barrier
    tile.TileContext._tail_patched = True


def _build_nc(w_scale):
    import concourse.bass as bass
    import concourse.mybir as mybir
    import concourse.tile as tile

    _patch_tile_tail()

    f32 = mybir.dt.float32
    bf16 = mybir.dt.bfloat16
    fp8 = mybir.dt.float8e4
    Exp = mybir.ActivationFunctionType.Exp

    in_dt = fp8
    exp_scale = 1.0 / w_scale
    n_warm = int(os.environ.get("KERNEL_WARM", "8"))

    nc = bass.Bass()
    # h interleaves [wt k-pair block (2KB) | et tile-0 k-pair block (1KB)]
    # per partition, so each k-pair group of batch tile 0 is gated by one
    # large-line transfer; k-pair-major weight order also serves tiles 1-7.
    h = nc.dram_tensor("h", [P, NK // 2, 3072], in_dt, kind="ExternalInput")
    et = nc.dram_tensor("et", [P, NB - 1, NK, 512], in_dt, kind="ExternalInput")
    mm = nc.dram_tensor("mm", [P, NV, NJ], in_dt, kind="ExternalInput")
    bias = nc.dram_tensor("bias", [P, NV], f32, kind="ExternalInput")
    # slot NB holds the second half-partial of the last batch tile (its
    # segment matmul is split in two to shorten the tail chain); the host
    # adds slots NB-1 and NB.
    out = nc.dram_tensor("out", [NJ, NB + 1, 512], bf16, kind="ExternalOutput")

    with (
        tile.TileContext(nc) as tc,
        tc.tile_pool(name="const", bufs=1) as cpool,
        tc.tile_pool(name="expp", bufs=3) as xpool,
        tc.tile_pool(name="ps", bufs=6, space="PSUM") as pspool,
        tc.tile_pool(name="ps2", bufs=2, space="PSUM") as ps2pool,
        tc.tile_pool(name="uo", bufs=2) as upool,
        tc.tile_pool(name="warm", bufs=1) as wpool,
    ):
        # All input triggers ride the Activation HWDGE ring in priority
        # order (the ring is FIFO at packet granularity across the 16 SDMA
        # engines). Output stores go on the otherwise-idle SP ring: an
        # HWDGE trigger waits at the *issuing sequencer*, which would stall
        # the exp pipeline if the stores were issued from Activation.
        h_sb = cpool.tile([P, NK // 2, 3072], in_dt)
        et_sb = cpool.tile([P, NB - 1, NK, 512], in_dt)
        b_sb = cpool.tile([P, NV], f32)
        m_sb = cpool.tile([P, NV, NJ], in_dt)
        nc.scalar.dma_start(h_sb[:, 0:2], h[:, 0:2])
        nc.scalar.dma_start(b_sb[:], bias[:])
        nc.scalar.dma_start(h_sb[:, 2:4], h[:, 2:4])
        nc.scalar.dma_start(et_sb[:, 0:1], et[:, 0:1])
        nc.scalar.dma_start(m_sb[:], mm[:])
        nc.scalar.dma_start(et_sb[:, 1:3], et[:, 1:3])
        nc.scalar.dma_start(et_sb[:, 3:7], et[:, 3:7])

        wt_kp = [
            h_sb[:, kp, 0:2048].rearrange("p (ko v q) -> p ko v q", ko=2, v=NV, q=P)
            for kp in range(NK // 2)
        ]
        et0_kp = [
            h_sb[:, kp, 2048:3072].rearrange("p (ko n) -> p ko n", ko=2, n=512)
            for kp in range(NK // 2)
        ]

        # PE HAM warm-up: the clock gate only opens after ~3.4us of
        # sustained PE activity; bridge the input-DMA window with throwaway
        # matmuls so batch tile 0 streams at 2.4 GHz.
        warm_sb = wpool.tile([P, 512], in_dt)
        nc.gpsimd.memset(warm_sb[:], 0)
        warm_act = wpool.tile([P, 16], in_dt)
        # pull the ~2.7us Exp table load into the DMA-wait window
        nc.scalar.activation(warm_act[:], warm_sb[:, 0:16], Exp)
        # borrow a slot from the main psum pool; it returns to the
        # rotation once the warm-up matmuls are done
        warm_ps = pspool.tile([P, 512], f32, tag="ps")
        for _ in range(n_warm):
            nc.tensor.matmul(
                warm_ps[:],
                lhsT=warm_sb[:, 0:P],
                rhs=warm_sb[:],
                start=True,
                stop=True,
            )

        def emit_mm2_mms(exp_t, vlo, vhi, ups):
            for v in range(vlo, vhi, 2):
                nc.tensor.matmul(
                    ups[:],
                    lhsT=m_sb[:, v : v + 2, :],
                    rhs=exp_t[:, v : v + 2, :],
                    start=(v == vlo),
                    stop=(v == vhi - 2),
                    perf_mode=mybir.MatmulPerfMode.DoubleRow,
                )

        def emit_mm2(n, exp_t):
            ups = ps2pool.tile([NJ, 512], f32, tag="ups")
            emit_mm2_mms(exp_t, 0, NV, ups)
            u_sb = upool.tile([NJ, 512], bf16, tag="u")
            nc.vector.tensor_copy(u_sb[:], ups[:])
            nc.sync.dma_start(out[:, n, :], u_sb[:])

        # Batch tile 0 runs k-pair-outer across all 8 PSUM banks: its first
        # matmuls need only the first h transfer, so real work starts while
        # the rest of the inputs stream in.
        exp0 = xpool.tile([P, NV, 512], in_dt, tag="exp")
        ps0 = [
            pspool.tile([P, 512], f32, tag="ps", name=f"ps0_{i}")
            for i in range(6)
        ] + [
            ps2pool.tile([P, 512], f32, tag="ups", name=f"ps0_{6 + i}")
            for i in range(2)
        ]
        for kp in range(NK // 2):
            rhs = et0_kp[kp]
            for v in range(NV):
                nc.tensor.matmul(
                    ps0[v][:],
                    lhsT=wt_kp[kp][:, :, v, :],
                    rhs=rhs,
                    start=(kp == 0),
                    stop=(kp == NK // 2 - 1),
                    perf_mode=mybir.MatmulPerfMode.DoubleRow,
                )
        for v in range(NV):
            nc.scalar.activation(
                exp0[:, v, :], ps0[v][:], Exp, bias=b_sb[:, v : v + 1],
                scale=exp_scale,
            )

        # MM2 of tile n is deferred to tile n+1's v==6 slot: tile 0's exps
        # drain through ScalarE in a ~5.5us burst, so an earlier slot would
        # park the PE queue on an exp-complete wait.
        pending = (0, exp0)
        for n in range(1, NB):
            exp_t = xpool.tile([P, NV, 512], in_dt, tag="exp")
            for v in range(NV):
                ps = pspool.tile([P, 512], f32, tag="ps")
                for k in range(0, NK, 2):
                    nc.tensor.matmul(
                        ps[:],
                        lhsT=wt_kp[k // 2][:, :, v, :],
                        rhs=et_sb[:, n - 1, k : k + 2, :],
                        start=(k == 0),
                        stop=(k == NK - 2),
                        perf_mode=mybir.MatmulPerfMode.DoubleRow,
                    )
                nc.scalar.activation(
                    exp_t[:, v, :], ps[:], Exp, bias=b_sb[:, v : v + 1], scale=exp_scale
                )
                if pending is not None and v == 6:
                    emit_mm2(*pending)
                    pending = None
            pending = (n, exp_t)

        # Last tile's MM2 in two host-summed halves so the final chain after
        # the last exp is only 2 matmuls + a cast + a 24KB store.
        n_last, exp_last = pending
        ups_a = ps2pool.tile([NJ, 512], f32, tag="ups")
        emit_mm2_mms(exp_last, 0, NV // 2, ups_a)
        u_a = upool.tile([NJ, 512], bf16, tag="u")
        nc.vector.tensor_copy(u_a[:], ups_a[:])
        nc.sync.dma_start(out[:, n_last, :], u_a[:])
        ups_b = ps2pool.tile([NJ, 512], f32, tag="ups")
        emit_mm2_mms(exp_last, NV // 2, NV, ups_b)
        u_b = upool.tile([NJ, 512], bf16, tag="u")
        nc.vector.tensor_copy(u_b[:], ups_b[:])
        nc.sync.dma_start(out[:, NB, 0:256], u_b[:, 0:256])
        nc.sync.dma_start(out[:, NB, 256:], u_b[:, 256:])
    _split_excess_waits(nc)
    return nc


def _install_ntff_hook():
    """bass_utils' axon trace path imports antenv.axon_hooks, absent in this
    image; shim it using trn_boot's ctypes NTFF hook."""
    if "antenv.axon_hooks" in sys.modules:
        return
    try:
        from trn_agent_boot.trn_boot import _ntff_profile_via_ctypes

        hook = _ntff_profile_via_ctypes("/opt/axon/libaxon_pjrt.so")
    except Exception:
        hook = None
    mod = types.ModuleType("antenv.axon_hooks")
    mod.get_axon_ntff_profile_hook = lambda: hook
    sys.modules["antenv.axon_hooks"] = mod


def kernel(embeddings, W, b, valid_states):
    global LAST_EXEC_NS, LAST_RESULT
    assert not USE_BF16, "bf16 fallback removed"
    E = np.asarray(embeddings, dtype=np.float32)
    Wf = np.asarray(W, dtype=np.float32)
    bf = np.asarray(b, dtype=np.float32)
    vs = np.asarray(valid_states).astype(np.int64)

    in_dt = ml_dtypes.float8_e4m3
    w_scale = _pick_w_scale(float(np.abs(Wf).max()))
    Wp = Wf * w_scale

    # etk[k, p, n, j] = E[n*512+j, k*128+p]
    etk = E.T.astype(in_dt).reshape(NK, P, NB, 512)
    # tiles 1..NB-1, 32KB-contiguous per partition
    et_host = np.ascontiguousarray(etk[:, :, 1:, :].transpose(1, 2, 0, 3))
    # et tile 0's k-pair blocks, interleaved into the head tensor below
    et0_blk = np.ascontiguousarray(
        etk[:, :, 0, :].reshape(NK // 2, 2, P, 512).transpose(2, 0, 1, 3)
    ).reshape(P, NK // 2, 1024)

    # One-hot segment matrix M [N_VALID, 48]
    M = np.zeros((N_VALID, NJ), dtype=in_dt)
    stride = N_TOTAL
    for c, n_i in enumerate(OUTCOMES):
        stride //= n_i
        digit = (vs // stride) % n_i
        M[np.arange(N_VALID), c * 8 + digit] = 1

    in_maps = []
    for core in range(N_CORES):
        sl = slice(core * V_S, (core + 1) * V_S)
        # wk[k, p, v, q] = W[v*128+q, k*128+p] * scale
        wk = Wp[sl, :].T.astype(in_dt).reshape(NK, P, NV, P)
        h_host = np.empty((P, NK // 2, 3072), dtype=in_dt)
        h_host[:, :, 0:2048] = (
            wk.reshape(NK // 2, 2, P, NV, P).transpose(2, 0, 1, 3, 4)
            .reshape(P, NK // 2, 2048)
        )
        h_host[:, :, 2048:3072] = et0_blk
        m_host = np.ascontiguousarray(M[sl].reshape(NV, P, NJ).transpose(1, 0, 2))
        b_host = np.ascontiguousarray(bf[sl].reshape(NV, P).T)
        in_maps.append(
            {"h": h_host, "et": et_host, "mm": m_host, "bias": b_host}
        )

    from concourse.bass_utils import run_bass_kernel_spmd

    key = w_scale
    if key not in _compiled_cache:
        _compiled_cache[key] = _build_nc(w_scale)
    nc_mod = _compiled_cache[key]

    kwargs = {}
    if os.environ.get("KERNEL_TRACE"):
        _install_ntff_hook()
        kwargs["trace"] = True

    res = run_bass_kernel_spmd(
        nc_mod, in_maps, core_ids=list(range(N_CORES)), **kwargs
    )
    LAST_EXEC_NS = res.exec_time_ns
    LAST_RESULT = res

    U = np.zeros((NJ, B), dtype=np.float64)
    for r in res.results:
        o = r["out"].astype(np.float64)  # [NJ, NB+1, 512]
        o[:, NB - 1] += o[:, NB]  # second half-partial of the last tile
        U += o[:, :NB].reshape(NJ, B)
    denom = U[0:8].sum(axis=0)  # [B] total softmax denominator
    marg = U.reshape(6, 8, B) / denom  # [6, 8, B]
    return np.ascontiguousarray(marg.transpose(0, 2, 1)).astype(np.float32)


# revision 14
# speedup vs baseline: 1.0398x; 1.0118x over previous
"""AdmissibleStatesHead on 8 Trainium2 NeuronCores.

marginals[c] = segment_sum(softmax(E @ W.T + b), digit_c)  ==  P @ M_c
where M is a one-hot [N_VALID, 48] matrix built on host from valid_states.

Device work per core (valid-states sharded 8 ways, batch replicated):
  logits^T tile [128v, 512b] = sum_k wt[k,v].T @ et[k,b]   (fp8 DoubleRow PE, fp32 PSUM)
  exp tile = Exp(logits^T * inv_scale + bias)              (ScalarE, PSUM -> SBUF fp8)
  U^T [48, 512b] += M_chunk.T @ exp_tile                   (fp8 DoubleRow PE)
Host: sum per-core partials, normalize by concept-0 bucket sum (= softmax
denominator), reshape to [6, B, 8]. W is pre-scaled by a power of two into
fp8's range; the Exp activation's free affine undoes it.

Set KERNEL_BF16=1 for a bf16 fallback (~1.8x slower, ~15x more accurate);
KERNEL_TRACE=1 captures an NTFF profile and fills LAST_EXEC_NS.
"""

import os
import sys
import types

import numpy as np
import ml_dtypes

OUTCOMES = [8, 8, 8, 8, 8, 8]
N_TOTAL = 262144
N_VALID = 8192
B, D = 4096, 1024
N_CORES = 8
P = 128
V_S = N_VALID // N_CORES  # 1024 valid states per core
NK = D // P               # 8 contraction chunks
NV = V_S // P             # 8 v-tiles per core
NB = B // 512             # 8 batch tiles of 512
NJ = 48                   # 6 concepts x 8 outcomes

# W values are small (~N(0, 0.02^2) per spec); scale into fp8e4m3's normal
# range and undo the scale for free inside the Exp activation. Chosen per
# call from the data as a power of two; the compiled module is cached per
# scale value.
DEFAULT_W_SCALE = 64.0

USE_BF16 = bool(os.environ.get("KERNEL_BF16"))

LAST_EXEC_NS = None
LAST_RESULT = None
_compiled_cache = {}


def _pick_w_scale(wmax):
    import math

    if not np.isfinite(wmax) or wmax <= 0:
        return DEFAULT_W_SCALE
    # keep max|W*scale| around <=192 (fp8e4m3 max 448), scale a power of 2
    s = 2.0 ** math.floor(math.log2(192.0 / wmax))
    return float(min(max(s, 2.0 ** -10), 2.0 ** 20))


def _split_excess_waits(nc, limit=1):
    """This walrus build rejects instructions carrying more than ~1 sync-wait
    ("Too many sync wait commands"). Hoist excess waits onto injected NoOps
    right before the instruction on the same engine — sequencers are in-order,
    so the semantics are identical."""
    import concourse.mybir as mybir

    ctr = 0
    main_bb = nc.m.functions[0].blocks[0]
    stripped = []
    for ins in main_bb.instructions:
        nm = str(ins.name)
        op = ins.concise_opcode()
        if op == "Drain" or (op == "EventSemaphore" and nm.startswith("barrier_")):
            continue
        stripped.append(ins)
    main_bb.instructions = stripped
    for fn in nc.m.functions:
        for bb in fn.blocks:
            insts = bb.instructions
            new = []
            changed = False
            for ins in insts:
                si = ins.sync_info
                lim = 1 if ins.concise_opcode() == "Drain" else limit
                if si is not None and len(si.on_wait) > lim:
                    waits = list(si.on_wait)
                    for w in waits[:-lim]:
                        ctr += 1
                        nop = mybir.InstNoOp(name=f"waitsplit_{ctr}", ins=[], outs=[])
                        nop.engine = ins.engine
                        nop.sync_info = mybir.SyncInfo(on_update=[], on_wait=[w])
                        new.append(nop)
                    ins.sync_info = mybir.SyncInfo(
                        on_update=list(si.on_update), on_wait=waits[-lim:]
                    )
                    changed = True
                new.append(ins)
            if changed:
                bb.instructions = new


def _patch_tile_tail():
    import concourse.tile as tile
    from concourse.vector_clock import ScopedClock

    if getattr(tile.TileContext, "_tail_patched", False):
        return

    def _drain_and_barrier(self, tick_clock, wait_clock):
        drain_inst = self.nc.sync.drain()
        wait_clock.add_sem_waits(
            drain_inst.ins, ScopedClock({None: tick_clock.global_clock})
        )
        self.nc.all_engine_barrier()
        popped = self.nc._tile_sem_poison_stack.pop()
        assert popped is self._sem_poison
        self.nc.clear_and_free_semaphores(list(self.sems.allocated().values()))

    tile.TileContext._drain_and_barrier = _drain_and_barrier
    tile.TileContext._tail_patched = True


def _build_nc(w_scale):
    import concourse.bass as bass
    import concourse.mybir as mybir
    import concourse.tile as tile

    _patch_tile_tail()

    f32 = mybir.dt.float32
    bf16 = mybir.dt.bfloat16
    fp8 = mybir.dt.float8e4
    Exp = mybir.ActivationFunctionType.Exp

    in_dt = fp8
    exp_scale = 1.0 / w_scale
    n_warm = int(os.environ.get("KERNEL_WARM", "7"))

    nc = bass.Bass()
    # h interleaves [wt k-pair block (2KB) | et tile-0 k-pair block (1KB)]
    # per partition, so each k-pair group of batch tile 0 is gated by one
    # large-line transfer; k-pair-major weight order also serves tiles 1-7.
    h = nc.dram_tensor("h", [P, NK // 2, 3072], in_dt, kind="ExternalInput")
    et = nc.dram_tensor("et", [P, NB - 1, NK, 512], in_dt, kind="ExternalInput")
    mm = nc.dram_tensor("mm", [P, NV, NJ], in_dt, kind="ExternalInput")
    bias = nc.dram_tensor("bias", [P, NV], f32, kind="ExternalInput")
    # slot NB holds the second half-partial of the last batch tile (its
    # segment matmul is split in two to shorten the tail chain); the host
    # adds slots NB-1 and NB.
    out = nc.dram_tensor("out", [NJ, NB + 1, 512], bf16, kind="ExternalOutput")

    with (
        tile.TileContext(nc) as tc,
        tc.tile_pool(name="const", bufs=1) as cpool,
        tc.tile_pool(name="expp", bufs=3) as xpool,
        tc.tile_pool(name="ps", bufs=6, space="PSUM") as pspool,
        tc.tile_pool(name="ps2", bufs=2, space="PSUM") as ps2pool,
        tc.tile_pool(name="uo", bufs=2) as upool,
        tc.tile_pool(name="warm", bufs=1) as wpool,
    ):
        # All input triggers ride the Activation HWDGE ring in priority
        # order (the ring is FIFO at packet granularity across the 16 SDMA
        # engines). Output stores go on the otherwise-idle SP ring: an
        # HWDGE trigger waits at the *issuing sequencer*, which would stall
        # the exp pipeline if the stores were issued from Activation.
        h_sb = cpool.tile([P, NK // 2, 3072], in_dt)
        et_sb = cpool.tile([P, NB - 1, NK, 512], in_dt)
        b_sb = cpool.tile([P, NV], f32)
        m_sb = cpool.tile([P, NV, NJ], in_dt)
        nc.scalar.dma_start(h_sb[:, 0], h[:, 0])
        nc.scalar.dma_start(b_sb[:], bias[:])
        nc.scalar.dma_start(h_sb[:, 1], h[:, 1])
        nc.scalar.dma_start(h_sb[:, 2], h[:, 2])
        nc.scalar.dma_start(h_sb[:, 3], h[:, 3])
        nc.scalar.dma_start(et_sb[:, 0:1], et[:, 0:1])
        nc.scalar.dma_start(m_sb[:], mm[:])
        nc.scalar.dma_start(et_sb[:, 1:3], et[:, 1:3])
        nc.scalar.dma_start(et_sb[:, 3:7], et[:, 3:7])

        wt_kp = [
            h_sb[:, kp, 0:2048].rearrange("p (ko v q) -> p ko v q", ko=2, v=NV, q=P)
            for kp in range(NK // 2)
        ]
        et0_kp = [
            h_sb[:, kp, 2048:3072].rearrange("p (ko n) -> p ko n", ko=2, n=512)
            for kp in range(NK // 2)
        ]

        # PE HAM warm-up: the clock gate only opens after ~3.4us of
        # sustained PE activity; bridge the input-DMA window with throwaway
        # matmuls so batch tile 0 streams at 2.4 GHz.
        warm_sb = wpool.tile([P, 512], in_dt)
        nc.gpsimd.memset(warm_sb[:], 0)
        warm_act = wpool.tile([P, 16], in_dt)
        # pull the ~2.7us Exp table load into the DMA-wait window
        nc.scalar.activation(warm_act[:], warm_sb[:, 0:16], Exp)
        # borrow a slot from the main psum pool; it returns to the
        # rotation once the warm-up matmuls are done
        warm_ps = pspool.tile([P, 512], f32, tag="ps")
        for _ in range(n_warm):
            nc.tensor.matmul(
                warm_ps[:],
                lhsT=warm_sb[:, 0:P],
                rhs=warm_sb[:],
                start=True,
                stop=True,
            )

        def emit_mm2_mms(exp_t, vlo, vhi, ups):
            for v in range(vlo, vhi, 2):
                nc.tensor.matmul(
                    ups[:],
                    lhsT=m_sb[:, v : v + 2, :],
                    rhs=exp_t[:, v : v + 2, :],
                    start=(v == vlo),
                    stop=(v == vhi - 2),
                    perf_mode=mybir.MatmulPerfMode.DoubleRow,
                )

        def emit_mm2(n, exp_t):
            ups = ps2pool.tile([NJ, 512], f32, tag="ups")
            emit_mm2_mms(exp_t, 0, NV, ups)
            u_sb = upool.tile([NJ, 512], bf16, tag="u")
            nc.vector.tensor_copy(u_sb[:], ups[:])
            nc.sync.dma_start(out[:, n, :], u_sb[:])

        # Batch tile 0 runs k-pair-outer across all 8 PSUM banks: its first
        # matmuls need only the first h transfer, so real work starts while
        # the rest of the inputs stream in.
        exp0 = xpool.tile([P, NV, 512], in_dt, tag="exp")
        ps0 = [
            pspool.tile([P, 512], f32, tag="ps", name=f"ps0_{i}")
            for i in range(6)
        ] + [
            ps2pool.tile([P, 512], f32, tag="ups", name=f"ps0_{6 + i}")
            for i in range(2)
        ]
        for kp in range(NK // 2):
            rhs = et0_kp[kp]
            for v in range(NV):
                nc.tensor.matmul(
                    ps0[v][:],
                    lhsT=wt_kp[kp][:, :, v, :],
                    rhs=rhs,
                    start=(kp == 0),
                    stop=(kp == NK // 2 - 1),
                    perf_mode=mybir.MatmulPerfMode.DoubleRow,
                )
        for v in range(NV):
            nc.scalar.activation(
                exp0[:, v, :], ps0[v][:], Exp, bias=b_sb[:, v : v + 1],
                scale=exp_scale,
            )

        # MM2 of tile n is deferred to tile n+1's v==6 slot: tile 0's exps
        # drain through ScalarE in a ~5.5us burst, so an earlier slot would
        # park the PE queue on an exp-complete wait.
        pending = (0, exp0)
        for n in range(1, NB):
            exp_t = xpool.tile([P, NV, 512], in_dt, tag="exp")
            for v in range(NV):
                ps = pspool.tile([P, 512], f32, tag="ps")
                for k in range(0, NK, 2):
                    nc.tensor.matmul(
                        ps[:],
                        lhsT=wt_kp[k // 2][:, :, v, :],
                        rhs=et_sb[:, n - 1, k : k + 2, :],
                        start=(k == 0),
                        stop=(k == NK - 2),
                        perf_mode=mybir.MatmulPerfMode.DoubleRow,
                    )
                nc.scalar.activation(
                    exp_t[:, v, :], ps[:], Exp, bias=b_sb[:, v : v + 1], scale=exp_scale
                )
                if pending is not None and v == 6:
                    emit_mm2(*pending)
                    pending = None
            pending = (n, exp_t)

        # Last tile's MM2 in two host-summed halves so the final chain after
        # the last exp is only 2 matmuls + a cast + a 24KB store.
        n_last, exp_last = pending
        ups_a = ps2pool.tile([NJ, 512], f32, tag="ups")
        emit_mm2_mms(exp_last, 0, NV // 2, ups_a)
        u_a = upool.tile([NJ, 512], bf16, tag="u")
        nc.vector.tensor_copy(u_a[:], ups_a[:])
        nc.sync.dma_start(out[:, n_last, :], u_a[:])
        ups_b = ps2pool.tile([NJ, 512], f32, tag="ups")
        emit_mm2_mms(exp_last, NV // 2, NV, ups_b)
        u_b = upool.tile([NJ, 512], bf16, tag="u")
        nc.vector.tensor_copy(u_b[:], ups_b[:])
        nc.sync.dma_start(out[:, NB, :], u_b[:])
    _split_excess_waits(nc)
    return nc


def _install_ntff_hook():
    """bass_utils' axon trace path imports antenv.axon_hooks, absent in this
    image; shim it using trn_boot's ctypes NTFF hook."""
    if "antenv.axon_hooks" in sys.modules:
        return
    try:
        from trn_agent_boot.trn_boot import _ntff_profile_via_ctypes

        hook = _ntff_profile_via_ctypes("/opt/axon/libaxon_pjrt.so")
    except Exception:
        hook = None
    mod = types.ModuleType("antenv.axon_hooks")
    mod.get_axon_ntff_profile_hook = lambda: hook
    sys.modules["antenv.axon_hooks"] = mod


def kernel(embeddings, W, b, valid_states):
    global LAST_EXEC_NS, LAST_RESULT
    assert not USE_BF16, "bf16 fallback removed"
    E = np.asarray(embeddings, dtype=np.float32)
    Wf = np.asarray(W, dtype=np.float32)
    bf = np.asarray(b, dtype=np.float32)
    vs = np.asarray(valid_states).astype(np.int64)

    in_dt = ml_dtypes.float8_e4m3
    w_scale = _pick_w_scale(float(np.abs(Wf).max()))
    Wp = Wf * w_scale

    # etk[k, p, n, j] = E[n*512+j, k*128+p]
    etk = E.T.astype(in_dt).reshape(NK, P, NB, 512)
    # tiles 1..NB-1, 32KB-contiguous per partition
    et_host = np.ascontiguousarray(etk[:, :, 1:, :].transpose(1, 2, 0, 3))
    # et tile 0's k-pair blocks, interleaved into the head tensor below
    et0_blk = np.ascontiguousarray(
        etk[:, :, 0, :].reshape(NK // 2, 2, P, 512).transpose(2, 0, 1, 3)
    ).reshape(P, NK // 2, 1024)

    # One-hot segment matrix M [N_VALID, 48]
    M = np.zeros((N_VALID, NJ), dtype=in_dt)
    stride = N_TOTAL
    for c, n_i in enumerate(OUTCOMES):
        stride //= n_i
        digit = (vs // stride) % n_i
        M[np.arange(N_VALID), c * 8 + digit] = 1

    in_maps = []
    for core in range(N_CORES):
        sl = slice(core * V_S, (core + 1) * V_S)
        # wk[k, p, v, q] = W[v*128+q, k*128+p] * scale
        wk = Wp[sl, :].T.astype(in_dt).reshape(NK, P, NV, P)
        h_host = np.empty((P, NK // 2, 3072), dtype=in_dt)
        h_host[:, :, 0:2048] = (
            wk.reshape(NK // 2, 2, P, NV, P).transpose(2, 0, 1, 3, 4)
            .reshape(P, NK // 2, 2048)
        )
        h_host[:, :, 2048:3072] = et0_blk
        m_host = np.ascontiguousarray(M[sl].reshape(NV, P, NJ).transpose(1, 0, 2))
        b_host = np.ascontiguousarray(bf[sl].reshape(NV, P).T)
        in_maps.append(
            {"h": h_host, "et": et_host, "mm": m_host, "bias": b_host}
        )

    from concourse.bass_utils import run_bass_kernel_spmd

    key = w_scale
    if key not in _compiled_cache:
        _compiled_cache[key] = _build_nc(w_scale)
    nc_mod = _compiled_cache[key]

    kwargs = {}
    if os.environ.get("KERNEL_TRACE"):
        _install_ntff_hook()
        kwargs["trace"] = True

    res = run_bass_kernel_spmd(
        nc_mod, in_maps, core_ids=list(range(N_CORES)), **kwargs
    )
    LAST_EXEC_NS = res.exec_time_ns
    LAST_RESULT = res

    U = np.zeros((NJ, B), dtype=np.float64)
    for r in res.results:
        o = r["out"].astype(np.float64)  # [NJ, NB+1, 512]
        o[:, NB - 1] += o[:, NB]  # second half-partial of the last tile
        U += o[:, :NB].reshape(NJ, B)
    denom = U[0:8].sum(axis=0)  # [B] total softmax denominator
    marg = U.reshape(6, 8, B) / denom  # [6, 8, B]
    return np.ascontiguousarray(marg.transpose(0, 2, 1)).astype(np.float32)
